# revision 4
# baseline (speedup 1.0000x reference)
"""Causal single-head attention (B=8, S=2048, D=512) on 8 TRN2 NeuronCores.

Strategy: data-parallel over the batch dim — one batch element per core.
Each core computes, for its batch element:
    Q = q @ Wq.T + bq ; K_core = k @ Wk.T ; V = v @ Wv.T + bv
    scores = Q @ K_core.T / sqrt(D)   (causal)
    out = softmax(scores) @ V
Notes on the math vs the reference:
  - bk drops out exactly: adding bk to K shifts every score row by a
    per-row constant, and softmax is invariant to per-row shifts.
  - softmax is computed without max-subtraction: scores are O(+-6) here
    so exp() cannot overflow/underflow in fp32.
  - bv is folded into the V projection; with late normalization
    out = (P_unnorm @ V) * (1/rowsum), the bias passes through exactly
    because rowsum is computed from the same unnormalized P.
On-device layout: q/k/v arrive host-pre-transposed as [D, S] so the
contraction dim sits on SBUF partitions; QT/KT live as [e, s], V as
[s, e]; score tiles are computed transposed ([s_k=128, s_q=512]) so the
P tiles feed the PV matmul as stationary operands directly. Row sums
come from an extra N=1 matmul against a ones vector. Only the lower
triangle of blocks is computed; the 16 diagonal 128x128 sub-tiles are
masked with a precomputed triangular 0/1 mask.
"""

import numpy as np

B, S, D, P = 8, 2048, 512, 128
EB = D // P  # e-blocks (4)
DC = D // P  # d-chunks (4)
NQB = S // P  # 128-row q-blocks (16)
QW = 512  # q window (score-tile free dim)
NQC = S // QW  # q-chunks (4)
N_CORES = 8

_CACHE = {}


def _build():
    import concourse.tile as tile
    from concourse import bacc, mybir
    from contextlib import ExitStack

    F32 = mybir.dt.float32
    F32R = mybir.dt.float32r
    AF = mybir.ActivationFunctionType

    nc = bacc.Bacc("TRN2", target_bir_lowering=False, debug=False)

    qT = nc.dram_tensor("qT", [D, S], F32R, kind="ExternalInput").ap()
    kT = nc.dram_tensor("kT", [D, S], F32R, kind="ExternalInput").ap()
    vT = nc.dram_tensor("vT", [D, S], F32R, kind="ExternalInput").ap()
    wqT = nc.dram_tensor("wqT", [D, D], F32R, kind="ExternalInput").ap()
    wkT = nc.dram_tensor("wkT", [D, D], F32R, kind="ExternalInput").ap()
    wvT = nc.dram_tensor("wvT", [D, D], F32R, kind="ExternalInput").ap()
    bq4 = nc.dram_tensor("bq4", [P, EB], F32, kind="ExternalInput").ap()
    bvb = nc.dram_tensor("bvb", [P, D], F32, kind="ExternalInput").ap()
    cm = nc.dram_tensor("cm", [P, P], F32R, kind="ExternalInput").ap()
    ones_d = nc.dram_tensor("ones_in", [P, 2], F32R, kind="ExternalInput").ap()
    out_d = nc.dram_tensor("out", [S, D], F32, kind="ExternalOutput").ap()

    with tile.TileContext(nc) as tc, ExitStack() as ctx:
        consts = ctx.enter_context(tc.tile_pool(name="consts", bufs=1))
        wpool = ctx.enter_context(tc.tile_pool(name="wpool", bufs=2))
        instream = ctx.enter_context(tc.tile_pool(name="instream", bufs=5))
        acts = ctx.enter_context(tc.tile_pool(name="acts", bufs=1))
        ptpool = ctx.enter_context(tc.tile_pool(name="ptpool", bufs=18))
        opool = ctx.enter_context(tc.tile_pool(name="opool", bufs=3))
        small = ctx.enter_context(tc.tile_pool(name="small", bufs=4))
        psmm = ctx.enter_context(tc.tile_pool(name="psmm", bufs=2, space="PSUM"))
        psout = ctx.enter_context(tc.tile_pool(name="psout", bufs=2, space="PSUM"))
        psrow = ctx.enter_context(tc.tile_pool(name="psrow", bufs=2, space="PSUM"))

        cmask = consts.tile([P, P], F32R)
        nc.sync.dma_start(out=cmask, in_=cm)
        bias_q = consts.tile([P, EB], F32)
        nc.sync.dma_start(out=bias_q, in_=bq4)
        bias_vb = consts.tile([P, D], F32)
        nc.sync.dma_start(out=bias_vb, in_=bvb)
        ones = consts.tile([P, 2], F32R)
        nc.sync.dma_start(out=ones, in_=ones_d)

        # persistent per-core activations
        qt_sb = acts.tile([P, EB, S], F32R, tag="qt")  # QT[e, s]
        kt_sb = acts.tile([P, EB, S], F32R, tag="kt")  # KT[e, s] (no bias)
        v_sb = acts.tile([P, NQB, D], F32R, tag="v")  # V[s, e] (+bv)

        # ---- Q/K projections: out[e, s] = sum_d W[e, d] x[s, d] (+ bias) ----
        for w_dram, x_dram, dst, bias in (
            (wqT, qT, qt_sb, bias_q),
            (wkT, kT, kt_sb, None),
        ):
            w_sb = wpool.tile([P, DC, D], F32R, tag="w")
            nc.sync.dma_start(out=w_sb, in_=w_dram.rearrange("(c p) e -> p c e", p=P))
            xts = []
            for dc in range(DC):
                xt = instream.tile([P, S], F32R, tag="in")
                nc.sync.dma_start(out=xt, in_=x_dram[dc * P : (dc + 1) * P, :])
                xts.append(xt)
            for ec in range(EB):
                for sc in range(NQC):
                    ps = psmm.tile([P, QW], F32, tag="mm")
                    for dc in range(DC):
                        nc.tensor.matmul(
                            ps,
                            w_sb[:, dc, ec * P : (ec + 1) * P],
                            xts[dc][:, sc * QW : (sc + 1) * QW],
                            start=(dc == 0),
                            stop=(dc == DC - 1),
                        )
                    dslc = dst[:, ec, sc * QW : (sc + 1) * QW]
                    if bias is not None:
                        nc.vector.tensor_scalar_add(dslc, ps, bias[:, ec : ec + 1])
                    else:
                        nc.vector.tensor_copy(dslc, ps)

        # ---- V projection: out[s, e] = sum_d v[s, d] W[e, d] + bv ----
        w_sb = wpool.tile([P, DC, D], F32R, tag="w")
        nc.sync.dma_start(out=w_sb, in_=wvT.rearrange("(c p) e -> p c e", p=P))
        vts = []
        for dc in range(DC):
            xt = instream.tile([P, S], F32R, tag="in")
            nc.sync.dma_start(out=xt, in_=vT[dc * P : (dc + 1) * P, :])
            vts.append(xt)
        for sb in range(NQB):
            ps = psmm.tile([P, QW], F32, tag="mm")
            for dc in range(DC):
                nc.tensor.matmul(
                    ps,
                    vts[dc][:, sb * P : (sb + 1) * P],
                    w_sb[:, dc, :],
                    start=(dc == 0),
                    stop=(dc == DC - 1),
                )
            nc.vector.tensor_add(v_sb[:, sb, :], ps, bias_vb)

        # ---- attention, per 512-wide q chunk ----
        inv_sqrt_d = float(1.0 / np.sqrt(D))
        for qc in range(NQC):
            nkb = 4 * qc + 4  # causal: k-blocks 0 .. 4qc+3
            pts = []
            for kb in range(nkb):
                ps = psmm.tile([P, QW], F32, tag="mm")
                for ec in range(EB):
                    nc.tensor.matmul(
                        ps,
                        kt_sb[:, ec, kb * P : (kb + 1) * P],
                        qt_sb[:, ec, qc * QW : (qc + 1) * QW],
                        start=(ec == 0),
                        stop=(ec == EB - 1),
                    )
                pt = ptpool.tile([P, QW], F32R, tag="pt")
                nc.scalar.activation(pt, ps, AF.Exp, scale=inv_sqrt_d)
                t = kb - 4 * qc
                if t >= 0:  # diagonal block: mask its triangular 128x128 sub-tile
                    nc.vector.tensor_mul(
                        pt[:, t * P : (t + 1) * P], pt[:, t * P : (t + 1) * P], cmask
                    )
                pts.append(pt)
            for j in range(4):
                qb = 4 * qc + j
                po = psout.tile([P, D], F32, tag="po")
                pr = psrow.tile([P, 2], F32, tag="pr")
                for kb in range(qb + 1):
                    lhsT = pts[kb][:, j * P : (j + 1) * P]
                    nc.tensor.matmul(
                        po, lhsT, v_sb[:, kb, :],
                        start=(kb == 0), stop=(kb == qb),
                    )
                    nc.tensor.matmul(
                        pr, lhsT, ones,
                        start=(kb == 0), stop=(kb == qb),
                    )
                rec = small.tile([P, 1], F32, tag="rec")
                nc.vector.reciprocal(rec, pr[:, 0:1])
                ot = opool.tile([P, D], F32, tag="ot")
                nc.vector.tensor_scalar_mul(ot, po, rec)
                nc.sync.dma_start(out=out_d[qb * P : (qb + 1) * P, :], in_=ot)

    nc.compile()
    return nc


def _get_nc():
    if "nc" not in _CACHE:
        _CACHE["nc"] = _build()
    return _CACHE["nc"]


def _make_in_maps(q, k, v, Wq, bq, Wk, Wv, bv):
    q = np.asarray(q, dtype=np.float32)
    k = np.asarray(k, dtype=np.float32)
    v = np.asarray(v, dtype=np.float32)
    wq_t = np.ascontiguousarray(np.asarray(Wq, dtype=np.float32).T)
    wk_t = np.ascontiguousarray(np.asarray(Wk, dtype=np.float32).T)
    wv_t = np.ascontiguousarray(np.asarray(Wv, dtype=np.float32).T)
    bq4 = np.ascontiguousarray(np.asarray(bq, dtype=np.float32).reshape(EB, P).T)
    bvb = np.ascontiguousarray(
        np.tile(np.asarray(bv, dtype=np.float32)[None, :], (P, 1))
    )
    cm = np.triu(np.ones((P, P), dtype=np.float32))  # cm[kk, qq] = qq >= kk
    in_maps = []
    for c in range(N_CORES):
        in_maps.append(
            {
                "qT": np.ascontiguousarray(q[c].T),
                "kT": np.ascontiguousarray(k[c].T),
                "vT": np.ascontiguousarray(v[c].T),
                "wqT": wq_t,
                "wkT": wk_t,
                "wvT": wv_t,
                "bq4": bq4,
                "bvb": bvb,
                "cm": cm,
                "ones_in": np.ones((P, 2), dtype=np.float32),
            }
        )
    return in_maps


def _run(in_maps, trace=False):
    from concourse.bass_utils import run_bass_kernel_spmd

    nc = _get_nc()
    res = run_bass_kernel_spmd(
        nc, in_maps, core_ids=list(range(N_CORES)), trace=trace
    )
    out = np.stack([res.results[c]["out"] for c in range(N_CORES)], axis=0)
    return out, res


def kernel(q, k, v, mask, Wq, bq, Wk, bk, Wv, bv):
    q = np.asarray(q, dtype=np.float32)
    assert q.shape == (B, S, D), f"unexpected q shape {q.shape}"
    in_maps = _make_in_maps(q, k, v, Wq, bq, Wk, Wv, bv)
    out, _ = _run(in_maps, trace=False)
    return out


# revision 7
# speedup vs baseline: 1.2539x; 1.2539x over previous
"""Causal single-head attention (B=8, S=2048, D=512) on 8 TRN2 NeuronCores.

Strategy: data-parallel over the batch dim — one batch element per core.
Each core computes, for its batch element:
    Q = q @ Wq.T + bq ; K_core = k @ Wk.T ; V = v @ Wv.T + bv
    scores = Q @ K_core.T / sqrt(D)   (causal)
    out = softmax(scores) @ V
Notes on the math vs the reference:
  - bk drops out exactly: adding bk to K shifts every score row by a
    per-row constant, and softmax is invariant to per-row shifts.
  - softmax is computed without max-subtraction: scores are O(+-6) here
    so exp() cannot overflow/underflow in fp32.
  - bv is folded into the V projection; with late normalization
    out = (P_unnorm @ V) * (1/rowsum), the bias passes through exactly
    because rowsum is computed from the same unnormalized P.
On-device layout: q/k/v arrive host-pre-transposed as [D, S] so the
contraction dim sits on SBUF partitions; QT/KT live as [e, s], V as
[s, e]; score tiles are computed transposed ([s_k=128, s_q=512]) so the
P tiles feed the PV matmul as stationary operands directly. Row sums
come from an extra N=1 matmul against a ones vector. Only the lower
triangle of blocks is computed; the 16 diagonal 128x128 sub-tiles are
masked with a precomputed triangular 0/1 mask.
"""

import numpy as np

B, S, D, P = 8, 2048, 512, 128
EB = D // P  # e-blocks (4)
DC = D // P  # d-chunks (4)
NQB = S // P  # 128-row q-blocks (16)
QW = 512  # q window (score-tile free dim)
NQC = S // QW  # q-chunks (4)
N_CORES = 8
MM_DTYPE = "bf16"  # "bf16" | "f32r" — dtype of all matmul operands

_CACHE = {}


def _build():
    import concourse.tile as tile
    from concourse import bacc, mybir
    from contextlib import ExitStack

    F32 = mybir.dt.float32
    F32R = mybir.dt.bfloat16 if MM_DTYPE == "bf16" else mybir.dt.float32r
    AF = mybir.ActivationFunctionType

    nc = bacc.Bacc("TRN2", target_bir_lowering=False, debug=False)

    qT = nc.dram_tensor("qT", [D, S], F32R, kind="ExternalInput").ap()
    kT = nc.dram_tensor("kT", [D, S], F32R, kind="ExternalInput").ap()
    vT = nc.dram_tensor("vT", [D, S], F32R, kind="ExternalInput").ap()
    wqT = nc.dram_tensor("wqT", [D, D], F32R, kind="ExternalInput").ap()
    wkT = nc.dram_tensor("wkT", [D, D], F32R, kind="ExternalInput").ap()
    wvT = nc.dram_tensor("wvT", [D, D], F32R, kind="ExternalInput").ap()
    bq4 = nc.dram_tensor("bq4", [P, EB], F32, kind="ExternalInput").ap()
    bvb = nc.dram_tensor("bvb", [P, D], F32, kind="ExternalInput").ap()
    cm = nc.dram_tensor("cm", [P, P], F32R, kind="ExternalInput").ap()
    ones_d = nc.dram_tensor("ones_in", [P, 2], F32R, kind="ExternalInput").ap()
    out_d = nc.dram_tensor("out", [S, D], F32, kind="ExternalOutput").ap()

    with tile.TileContext(nc) as tc, ExitStack() as ctx:
        consts = ctx.enter_context(tc.tile_pool(name="consts", bufs=1))
        wpool = ctx.enter_context(tc.tile_pool(name="wpool", bufs=2))
        instream = ctx.enter_context(tc.tile_pool(name="instream", bufs=5))
        acts = ctx.enter_context(tc.tile_pool(name="acts", bufs=1))
        ptpool = ctx.enter_context(tc.tile_pool(name="ptpool", bufs=18))
        opool = ctx.enter_context(tc.tile_pool(name="opool", bufs=2))
        small = ctx.enter_context(tc.tile_pool(name="small", bufs=4))
        psmm = ctx.enter_context(tc.tile_pool(name="psmm", bufs=3, space="PSUM"))
        psout = ctx.enter_context(tc.tile_pool(name="psout", bufs=2, space="PSUM"))
        psrow = ctx.enter_context(tc.tile_pool(name="psrow", bufs=2, space="PSUM"))

        cmask = consts.tile([P, P], F32R)
        nc.sync.dma_start(out=cmask, in_=cm)
        bias_q = consts.tile([P, EB], F32)
        nc.sync.dma_start(out=bias_q, in_=bq4)
        bias_vb = consts.tile([P, D], F32)
        nc.sync.dma_start(out=bias_vb, in_=bvb)
        ones = consts.tile([P, 2], F32R)
        nc.sync.dma_start(out=ones, in_=ones_d)

        # persistent per-core activations
        qt_sb = acts.tile([P, EB, S], F32R, tag="qt")  # QT[e, s]
        kt_sb = acts.tile([P, EB, S], F32R, tag="kt")  # KT[e, s] (no bias)
        v_sb = acts.tile([P, NQB, D], F32R, tag="v")  # V[s, e] (+bv)

        # ---- Q/K projections: out[e, s] = sum_d W[e, d] x[s, d] (+ bias) ----
        for w_dram, x_dram, dst, bias in (
            (wqT, qT, qt_sb, bias_q),
            (wkT, kT, kt_sb, None),
        ):
            w_sb = wpool.tile([P, DC, D], F32R, tag="w")
            w_r = w_dram.rearrange("(c p) e -> p c e", p=P)
            nc.sync.dma_start(out=w_sb[:, :, :P], in_=w_r[:, :, :P])
            nc.sync.dma_start(out=w_sb[:, :, P:], in_=w_r[:, :, P:])
            xts = []
            for dc in range(DC):
                xt = instream.tile([P, S], F32R, tag="in")
                nc.sync.dma_start(
                    out=xt[:, :QW], in_=x_dram[dc * P : (dc + 1) * P, :QW]
                )
                xts.append(xt)
            for dc in range(DC):
                nc.sync.dma_start(
                    out=xts[dc][:, QW:], in_=x_dram[dc * P : (dc + 1) * P, QW:]
                )
            for ec in range(EB):
                for sc in range(NQC):
                    ps = psmm.tile([P, QW], F32, tag="mm")
                    for dc in range(DC):
                        nc.tensor.matmul(
                            ps,
                            w_sb[:, dc, ec * P : (ec + 1) * P],
                            xts[dc][:, sc * QW : (sc + 1) * QW],
                            start=(dc == 0),
                            stop=(dc == DC - 1),
                        )
                    dslc = dst[:, ec, sc * QW : (sc + 1) * QW]
                    if bias is not None:
                        nc.vector.tensor_scalar_add(dslc, ps, bias[:, ec : ec + 1])
                    else:
                        nc.vector.tensor_copy(dslc, ps)

        # ---- V projection: out[s, e] = sum_d v[s, d] W[e, d] + bv ----
        w_sb = wpool.tile([P, DC, D], F32R, tag="w")
        w_r = wvT.rearrange("(c p) e -> p c e", p=P)
        nc.sync.dma_start(out=w_sb[:, :, :P], in_=w_r[:, :, :P])
        nc.sync.dma_start(out=w_sb[:, :, P:], in_=w_r[:, :, P:])
        vts = []
        for dc in range(DC):
            xt = instream.tile([P, S], F32R, tag="in")
            nc.sync.dma_start(out=xt, in_=vT[dc * P : (dc + 1) * P, :])
            vts.append(xt)
        for sb in range(NQB):
            ps = psmm.tile([P, QW], F32, tag="mm")
            for dc in range(DC):
                nc.tensor.matmul(
                    ps,
                    vts[dc][:, sb * P : (sb + 1) * P],
                    w_sb[:, dc, :],
                    start=(dc == 0),
                    stop=(dc == DC - 1),
                )
            nc.vector.tensor_add(v_sb[:, sb, :], ps, bias_vb)

        # ---- attention, per 512-wide q chunk ----
        inv_sqrt_d = float(1.0 / np.sqrt(D))
        for qc in range(NQC):
            nkb = 4 * qc + 4  # causal: k-blocks 0 .. 4qc+3
            pts = []
            for kb in range(nkb):
                ps = psmm.tile([P, QW], F32, tag="mm")
                for ec in range(EB):
                    nc.tensor.matmul(
                        ps,
                        kt_sb[:, ec, kb * P : (kb + 1) * P],
                        qt_sb[:, ec, qc * QW : (qc + 1) * QW],
                        start=(ec == 0),
                        stop=(ec == EB - 1),
                    )
                pt = ptpool.tile([P, QW], F32R, tag="pt")
                nc.scalar.activation(pt, ps, AF.Exp, scale=inv_sqrt_d)
                t = kb - 4 * qc
                if t >= 0:  # diagonal block: mask its triangular 128x128 sub-tile
                    nc.vector.tensor_mul(
                        pt[:, t * P : (t + 1) * P], pt[:, t * P : (t + 1) * P], cmask
                    )
                pts.append(pt)
            og = opool.tile([P, 4, D], F32, tag="ot")
            for j in range(4):
                qb = 4 * qc + j
                po = psout.tile([P, D], F32, tag="po")
                pr = psrow.tile([P, 2], F32, tag="pr")
                for kb in range(qb + 1):
                    lhsT = pts[kb][:, j * P : (j + 1) * P]
                    nc.tensor.matmul(
                        po, lhsT, v_sb[:, kb, :],
                        start=(kb == 0), stop=(kb == qb),
                    )
                    nc.tensor.matmul(
                        pr, lhsT, ones,
                        start=(kb == 0), stop=(kb == qb),
                    )
                rec = small.tile([P, 1], F32, tag="rec")
                nc.vector.reciprocal(rec, pr[:, 0:1])
                nc.vector.tensor_scalar_mul(og[:, j, :], po, rec)
            nc.sync.dma_start(
                out=out_d[qc * QW : (qc + 1) * QW, :].rearrange(
                    "(c p) e -> p c e", p=P
                ),
                in_=og,
            )

    nc.compile()
    return nc


def _get_nc():
    if "nc" not in _CACHE:
        _CACHE["nc"] = _build()
    return _CACHE["nc"]


def _make_in_maps(q, k, v, Wq, bq, Wk, Wv, bv):
    import ml_dtypes

    mdt = ml_dtypes.bfloat16 if MM_DTYPE == "bf16" else np.float32
    q = np.asarray(q, dtype=np.float32)
    k = np.asarray(k, dtype=np.float32)
    v = np.asarray(v, dtype=np.float32)
    wq_t = np.ascontiguousarray(np.asarray(Wq, dtype=np.float32).T).astype(mdt)
    wk_t = np.ascontiguousarray(np.asarray(Wk, dtype=np.float32).T).astype(mdt)
    wv_t = np.ascontiguousarray(np.asarray(Wv, dtype=np.float32).T).astype(mdt)
    bq4 = np.ascontiguousarray(np.asarray(bq, dtype=np.float32).reshape(EB, P).T)
    bvb = np.ascontiguousarray(
        np.tile(np.asarray(bv, dtype=np.float32)[None, :], (P, 1))
    )
    cm = np.triu(np.ones((P, P), dtype=np.float32)).astype(mdt)  # cm[kk,qq]=qq>=kk
    in_maps = []
    for c in range(N_CORES):
        in_maps.append(
            {
                "qT": np.ascontiguousarray(q[c].T).astype(mdt),
                "kT": np.ascontiguousarray(k[c].T).astype(mdt),
                "vT": np.ascontiguousarray(v[c].T).astype(mdt),
                "wqT": wq_t,
                "wkT": wk_t,
                "wvT": wv_t,
                "bq4": bq4,
                "bvb": bvb,
                "cm": cm,
                "ones_in": np.ones((P, 2), dtype=mdt),
            }
        )
    return in_maps


def _run(in_maps, trace=False):
    from concourse.bass_utils import run_bass_kernel_spmd

    nc = _get_nc()
    res = run_bass_kernel_spmd(
        nc, in_maps, core_ids=list(range(N_CORES)), trace=trace
    )
    out = np.stack([res.results[c]["out"] for c in range(N_CORES)], axis=0)
    return out, res


def kernel(q, k, v, mask, Wq, bq, Wk, bk, Wv, bv):
    q = np.asarray(q, dtype=np.float32)
    assert q.shape == (B, S, D), f"unexpected q shape {q.shape}"
    in_maps = _make_in_maps(q, k, v, Wq, bq, Wk, Wv, bv)
    out, _ = _run(in_maps, trace=False)
    return out


# revision 8
# speedup vs baseline: 1.3162x; 1.0497x over previous
"""Causal single-head attention (B=8, S=2048, D=512) on 8 TRN2 NeuronCores.

Strategy: data-parallel over the batch dim — one batch element per core.
Each core computes, for its batch element:
    Q = q @ Wq.T + bq ; K_core = k @ Wk.T ; V = v @ Wv.T + bv
    scores = Q @ K_core.T / sqrt(D)   (causal)
    out = softmax(scores) @ V
Notes on the math vs the reference:
  - bk drops out exactly: adding bk to K shifts every score row by a
    per-row constant, and softmax is invariant to per-row shifts.
  - softmax is computed without max-subtraction: scores are O(+-6) here
    so exp() cannot overflow/underflow in fp32.
  - bv is folded into the V projection; with late normalization
    out = (P_unnorm @ V) * (1/rowsum), the bias passes through exactly
    because rowsum is computed from the same unnormalized P.
On-device layout: q/k/v arrive host-pre-transposed as [D, S] so the
contraction dim sits on SBUF partitions; QT/KT live as [e, s], V as
[s, e]; score tiles are computed transposed ([s_k=128, s_q=512]) so the
P tiles feed the PV matmul as stationary operands directly. Row sums
come from an extra N=1 matmul against a ones vector. Only the lower
triangle of blocks is computed; the 16 diagonal 128x128 sub-tiles are
masked with a precomputed triangular 0/1 mask.
"""

import numpy as np

B, S, D, P = 8, 2048, 512, 128
EB = D // P  # e-blocks (4)
DC = D // P  # d-chunks (4)
NQB = S // P  # 128-row q-blocks (16)
QW = 512  # q window (score-tile free dim)
NQC = S // QW  # q-chunks (4)
N_CORES = 8
MM_DTYPE = "bf16"  # "bf16" | "f32r" — dtype of all matmul operands

_CACHE = {}


def _build():
    import concourse.tile as tile
    from concourse import bacc, mybir
    from contextlib import ExitStack

    F32 = mybir.dt.float32
    F32R = mybir.dt.bfloat16 if MM_DTYPE == "bf16" else mybir.dt.float32r
    AF = mybir.ActivationFunctionType

    nc = bacc.Bacc("TRN2", target_bir_lowering=False, debug=False)

    qT = nc.dram_tensor("qT", [D, S], F32R, kind="ExternalInput").ap()
    kT = nc.dram_tensor("kT", [D, S], F32R, kind="ExternalInput").ap()
    vT = nc.dram_tensor("vT", [D, S], F32R, kind="ExternalInput").ap()
    wqT = nc.dram_tensor("wqT", [D, D], F32R, kind="ExternalInput").ap()
    wkT = nc.dram_tensor("wkT", [D, D], F32R, kind="ExternalInput").ap()
    wvT = nc.dram_tensor("wvT", [D, D], F32R, kind="ExternalInput").ap()
    bq4 = nc.dram_tensor("bq4", [P, EB], F32, kind="ExternalInput").ap()
    bvb = nc.dram_tensor("bvb", [P, D], F32, kind="ExternalInput").ap()
    cm = nc.dram_tensor("cm", [P, P], F32R, kind="ExternalInput").ap()
    ones_d = nc.dram_tensor("ones_in", [P, 2], F32R, kind="ExternalInput").ap()
    out_d = nc.dram_tensor("out", [S, D], F32, kind="ExternalOutput").ap()

    with tile.TileContext(nc) as tc, ExitStack() as ctx:
        consts = ctx.enter_context(tc.tile_pool(name="consts", bufs=1))
        wpool = ctx.enter_context(tc.tile_pool(name="wpool", bufs=2))
        instream = ctx.enter_context(tc.tile_pool(name="instream", bufs=5))
        acts = ctx.enter_context(tc.tile_pool(name="acts", bufs=1))
        ptpool = ctx.enter_context(tc.tile_pool(name="ptpool", bufs=18))
        opool = ctx.enter_context(tc.tile_pool(name="opool", bufs=2))
        small = ctx.enter_context(tc.tile_pool(name="small", bufs=4))
        psmm = ctx.enter_context(tc.tile_pool(name="psmm", bufs=4, space="PSUM"))
        psout = ctx.enter_context(tc.tile_pool(name="psout", bufs=2, space="PSUM"))
        psrow = ctx.enter_context(tc.tile_pool(name="psrow", bufs=2, space="PSUM"))

        cmask = consts.tile([P, P], F32R)
        bias_q = consts.tile([P, EB], F32)
        bias_vb = consts.tile([P, D], F32)
        ones = consts.tile([P, 2], F32R)

        # persistent per-core activations
        qt_sb = acts.tile([P, EB, S], F32R, tag="qt")  # QT[e, s]
        kt_sb = acts.tile([P, EB, S], F32R, tag="kt")  # KT[e, s] (no bias)
        v_sb = acts.tile([P, NQB, D], F32R, tag="v")  # V[s, e] (+bv)

        # ---- Q/K projections: out[e, s] = sum_d W[e, d] x[s, d] (+ bias) ----
        for w_dram, x_dram, dst, bias in (
            (wqT, qT, qt_sb, bias_q),
            (wkT, kT, kt_sb, None),
        ):
            w_sb = wpool.tile([P, DC, D], F32R, tag="w")
            w_r = w_dram.rearrange("(c p) e -> p c e", p=P)
            nc.sync.dma_start(out=w_sb[:, :, :P], in_=w_r[:, :, :P])
            nc.sync.dma_start(out=w_sb[:, :, P:], in_=w_r[:, :, P:])
            xts = []
            for dc in range(DC):
                xt = instream.tile([P, S], F32R, tag="in")
                nc.sync.dma_start(
                    out=xt[:, :QW], in_=x_dram[dc * P : (dc + 1) * P, :QW]
                )
                xts.append(xt)
            for dc in range(DC):
                nc.sync.dma_start(
                    out=xts[dc][:, QW:], in_=x_dram[dc * P : (dc + 1) * P, QW:]
                )
            if bias is not None:  # first pass: queue const DMAs after Q input
                nc.sync.dma_start(out=cmask, in_=cm)
                nc.sync.dma_start(out=bias_q, in_=bq4)
                nc.sync.dma_start(out=bias_vb, in_=bvb)
                nc.sync.dma_start(out=ones, in_=ones_d)
            for ec in range(EB):
                for sc in range(NQC):
                    ps = psmm.tile([P, QW], F32, tag="mm")
                    for dc in range(DC):
                        nc.tensor.matmul(
                            ps,
                            w_sb[:, dc, ec * P : (ec + 1) * P],
                            xts[dc][:, sc * QW : (sc + 1) * QW],
                            start=(dc == 0),
                            stop=(dc == DC - 1),
                        )
                    dslc = dst[:, ec, sc * QW : (sc + 1) * QW]
                    if bias is not None:
                        nc.vector.tensor_scalar_add(dslc, ps, bias[:, ec : ec + 1])
                    else:
                        nc.scalar.copy(dslc, ps)

        # ---- V projection: out[s, e] = sum_d v[s, d] W[e, d] + bv ----
        w_sb = wpool.tile([P, DC, D], F32R, tag="w")
        w_r = wvT.rearrange("(c p) e -> p c e", p=P)
        nc.sync.dma_start(out=w_sb[:, :, :P], in_=w_r[:, :, :P])
        nc.sync.dma_start(out=w_sb[:, :, P:], in_=w_r[:, :, P:])
        vts = []
        for dc in range(DC):
            xt = instream.tile([P, S], F32R, tag="in")
            nc.sync.dma_start(out=xt, in_=vT[dc * P : (dc + 1) * P, :])
            vts.append(xt)
        for sb in range(NQB):
            ps = psmm.tile([P, QW], F32, tag="mm")
            for dc in range(DC):
                nc.tensor.matmul(
                    ps,
                    vts[dc][:, sb * P : (sb + 1) * P],
                    w_sb[:, dc, :],
                    start=(dc == 0),
                    stop=(dc == DC - 1),
                )
            nc.vector.tensor_add(v_sb[:, sb, :], ps, bias_vb)

        # ---- attention, per 512-wide q chunk ----
        inv_sqrt_d = float(1.0 / np.sqrt(D))
        for qc in range(NQC):
            nkb = 4 * qc + 4  # causal: k-blocks 0 .. 4qc+3
            pts = []
            for kb in range(nkb):
                t = kb - 4 * qc  # >=0 only for the diagonal group
                off = max(0, t) * P  # columns below the diagonal are never read
                ps = psmm.tile([P, QW], F32, tag="mm")
                for ec in range(EB):
                    nc.tensor.matmul(
                        ps[:, off:],
                        kt_sb[:, ec, kb * P : (kb + 1) * P],
                        qt_sb[:, ec, qc * QW + off : (qc + 1) * QW],
                        start=(ec == 0),
                        stop=(ec == EB - 1),
                    )
                pt = ptpool.tile([P, QW], F32R, tag="pt")
                nc.scalar.activation(pt[:, off:], ps[:, off:], AF.Exp, scale=inv_sqrt_d)
                if t >= 0:  # diagonal block: mask its triangular 128x128 sub-tile
                    nc.vector.tensor_mul(
                        pt[:, off : off + P], pt[:, off : off + P], cmask
                    )
                pts.append(pt)
            og = opool.tile([P, 4, D], F32, tag="ot")
            for j in range(4):
                qb = 4 * qc + j
                po = psout.tile([P, D], F32, tag="po")
                pr = psrow.tile([P, 2], F32, tag="pr")
                for kb in range(qb + 1):
                    lhsT = pts[kb][:, j * P : (j + 1) * P]
                    nc.tensor.matmul(
                        po, lhsT, v_sb[:, kb, :],
                        start=(kb == 0), stop=(kb == qb),
                    )
                    nc.tensor.matmul(
                        pr, lhsT, ones,
                        start=(kb == 0), stop=(kb == qb),
                    )
                rec = small.tile([P, 1], F32, tag="rec")
                nc.vector.reciprocal(rec, pr[:, 0:1])
                nc.vector.tensor_scalar_mul(og[:, j, :], po, rec)
            nc.sync.dma_start(
                out=out_d[qc * QW : (qc + 1) * QW, :].rearrange(
                    "(c p) e -> p c e", p=P
                ),
                in_=og,
            )

    nc.compile()
    return nc


def _get_nc():
    if "nc" not in _CACHE:
        _CACHE["nc"] = _build()
    return _CACHE["nc"]


def _make_in_maps(q, k, v, Wq, bq, Wk, Wv, bv):
    import ml_dtypes

    mdt = ml_dtypes.bfloat16 if MM_DTYPE == "bf16" else np.float32
    q = np.asarray(q, dtype=np.float32)
    k = np.asarray(k, dtype=np.float32)
    v = np.asarray(v, dtype=np.float32)
    wq_t = np.ascontiguousarray(np.asarray(Wq, dtype=np.float32).T).astype(mdt)
    wk_t = np.ascontiguousarray(np.asarray(Wk, dtype=np.float32).T).astype(mdt)
    wv_t = np.ascontiguousarray(np.asarray(Wv, dtype=np.float32).T).astype(mdt)
    bq4 = np.ascontiguousarray(np.asarray(bq, dtype=np.float32).reshape(EB, P).T)
    bvb = np.ascontiguousarray(
        np.tile(np.asarray(bv, dtype=np.float32)[None, :], (P, 1))
    )
    cm = np.triu(np.ones((P, P), dtype=np.float32)).astype(mdt)  # cm[kk,qq]=qq>=kk
    in_maps = []
    for c in range(N_CORES):
        in_maps.append(
            {
                "qT": np.ascontiguousarray(q[c].T).astype(mdt),
                "kT": np.ascontiguousarray(k[c].T).astype(mdt),
                "vT": np.ascontiguousarray(v[c].T).astype(mdt),
                "wqT": wq_t,
                "wkT": wk_t,
                "wvT": wv_t,
                "bq4": bq4,
                "bvb": bvb,
                "cm": cm,
                "ones_in": np.ones((P, 2), dtype=mdt),
            }
        )
    return in_maps


def _run(in_maps, trace=False):
    from concourse.bass_utils import run_bass_kernel_spmd

    nc = _get_nc()
    res = run_bass_kernel_spmd(
        nc, in_maps, core_ids=list(range(N_CORES)), trace=trace
    )
    out = np.stack([res.results[c]["out"] for c in range(N_CORES)], axis=0)
    return out, res


def kernel(q, k, v, mask, Wq, bq, Wk, bk, Wv, bv):
    q = np.asarray(q, dtype=np.float32)
    assert q.shape == (B, S, D), f"unexpected q shape {q.shape}"
    in_maps = _make_in_maps(q, k, v, Wq, bq, Wk, Wv, bv)
    out, _ = _run(in_maps, trace=False)
    return out


# revision 9
# speedup vs baseline: 1.4538x; 1.1045x over previous
"""Causal single-head attention (B=8, S=2048, D=512) on 8 TRN2 NeuronCores.

Strategy: data-parallel over the batch dim — one batch element per core.
Each core computes, for its batch element:
    Q = q @ Wq.T + bq ; K_core = k @ Wk.T ; V = v @ Wv.T + bv
    scores = Q @ K_core.T / sqrt(D)   (causal)
    out = softmax(scores) @ V
Notes on the math vs the reference:
  - bk drops out exactly: adding bk to K shifts every score row by a
    per-row constant, and softmax is invariant to per-row shifts.
  - softmax is computed without max-subtraction: scores are O(+-6) here
    so exp() cannot overflow/underflow in fp32.
  - bv is folded into the V projection; with late normalization
    out = (P_unnorm @ V) * (1/rowsum), the bias passes through exactly
    because rowsum is computed from the same unnormalized P.
On-device layout: q/k/v arrive host-pre-transposed as [D, S] so the
contraction dim sits on SBUF partitions; QT/KT live as [e, s], V as
[s, e]; score tiles are computed transposed ([s_k=128, s_q<=512]) so
the P tiles feed the PV matmul as stationary operands directly. Row
sums come from an extra N=2 matmul against a ones vector. Only the
lower-triangular 128-column blocks are computed; the 16 diagonal
128x128 sub-tiles are masked with a precomputed triangular 0/1 mask.
Matmul operands are bf16 (PSUM accumulation and the softmax
normalization stay fp32).
"""

import numpy as np

B, S, D, P = 8, 2048, 512, 128
EB = D // P  # e-blocks (4)
DC = D // P  # d-chunks (4)
NQB = S // P  # 128-row q-blocks (16)
QW = 512  # q window (score-tile free dim)
NQC = S // QW  # q-chunks (4)
N_CORES = 8
MM_DTYPE = "bf16"  # "bf16" | "f32r" — dtype of all matmul operands

_CACHE = {}


def _build():
    import concourse.tile as tile
    from concourse import bacc, mybir
    from contextlib import ExitStack

    F32 = mybir.dt.float32
    MDT = mybir.dt.bfloat16 if MM_DTYPE == "bf16" else mybir.dt.float32r
    AF = mybir.ActivationFunctionType

    nc = bacc.Bacc("TRN2", target_bir_lowering=False, debug=False)

    qT = nc.dram_tensor("qT", [D, S], MDT, kind="ExternalInput").ap()
    kT = nc.dram_tensor("kT", [D, S], MDT, kind="ExternalInput").ap()
    vT = nc.dram_tensor("vT", [D, S], MDT, kind="ExternalInput").ap()
    wqT = nc.dram_tensor("wqT", [D, D], MDT, kind="ExternalInput").ap()
    wkT = nc.dram_tensor("wkT", [D, D], MDT, kind="ExternalInput").ap()
    wvT = nc.dram_tensor("wvT", [D, D], MDT, kind="ExternalInput").ap()
    bq4 = nc.dram_tensor("bq4", [P, EB], F32, kind="ExternalInput").ap()
    bvb = nc.dram_tensor("bvb", [P, D], F32, kind="ExternalInput").ap()
    cm = nc.dram_tensor("cm", [P, P], MDT, kind="ExternalInput").ap()
    ones_d = nc.dram_tensor("ones_in", [P, 2], MDT, kind="ExternalInput").ap()
    out_d = nc.dram_tensor("out", [S, D], F32, kind="ExternalOutput").ap()

    with tile.TileContext(nc) as tc, ExitStack() as ctx:
        consts = ctx.enter_context(tc.tile_pool(name="consts", bufs=1))
        wpool = ctx.enter_context(tc.tile_pool(name="wpool", bufs=2))
        instream = ctx.enter_context(tc.tile_pool(name="instream", bufs=3))
        acts = ctx.enter_context(tc.tile_pool(name="acts", bufs=1))
        ptpool = ctx.enter_context(tc.tile_pool(name="ptpool", bufs=18))
        opool = ctx.enter_context(tc.tile_pool(name="opool", bufs=2))
        small = ctx.enter_context(tc.tile_pool(name="small", bufs=4))
        psmm = ctx.enter_context(tc.tile_pool(name="psmm", bufs=4, space="PSUM"))
        psout = ctx.enter_context(tc.tile_pool(name="psout", bufs=2, space="PSUM"))
        psrow = ctx.enter_context(tc.tile_pool(name="psrow", bufs=2, space="PSUM"))

        cmask = consts.tile([P, P], MDT)
        bias_q = consts.tile([P, EB], F32)
        bias_vb = consts.tile([P, D], F32)
        ones = consts.tile([P, 2], MDT)

        # persistent per-core activations
        qt_sb = acts.tile([P, EB, S], MDT, tag="qt")  # QT[e, s]
        kt_sb = acts.tile([P, EB, S], MDT, tag="kt")  # KT[e, s] (no bias)
        v_sb = acts.tile([P, NQB, D], MDT, tag="v")  # V[s, e] (+bv)

        # ---- Q/K projections: out[e, s] = sum_d W[e, d] x[s, d] (+ bias) ----
        for w_dram, x_dram, dst, bias in (
            (wqT, qT, qt_sb, bias_q),
            (wkT, kT, kt_sb, None),
        ):
            w_sb = wpool.tile([P, DC, D], MDT, tag="w")
            w_r = w_dram.rearrange("(c p) e -> p c e", p=P)
            x_r = x_dram.rearrange("(c p) s -> p c s", p=P)
            xt = instream.tile([P, DC, S], MDT, tag="in")
            if bias is not None:
                # Q path: fine-grained first-need pieces so PE starts early
                nc.sync.dma_start(out=w_sb[:, :, :P], in_=w_r[:, :, :P])
                for sc in range(NQC):
                    nc.sync.dma_start(
                        out=xt[:, :, sc * QW : (sc + 1) * QW],
                        in_=x_r[:, :, sc * QW : (sc + 1) * QW],
                    )
                nc.sync.dma_start(out=w_sb[:, :, P:], in_=w_r[:, :, P:])
                # consts queued after the critical Q pieces
                nc.sync.dma_start(out=cmask, in_=cm)
                nc.sync.dma_start(out=bias_q, in_=bq4)
                nc.sync.dma_start(out=bias_vb, in_=bvb)
                nc.sync.dma_start(out=ones, in_=ones_d)
            else:
                nc.sync.dma_start(out=w_sb, in_=w_r)
                nc.sync.dma_start(out=xt, in_=x_r)
            for ec in range(EB):
                for sc in range(NQC):
                    ps = psmm.tile([P, QW], F32, tag="mm")
                    for dc in range(DC):
                        nc.tensor.matmul(
                            ps,
                            w_sb[:, dc, ec * P : (ec + 1) * P],
                            xt[:, dc, sc * QW : (sc + 1) * QW],
                            start=(dc == 0),
                            stop=(dc == DC - 1),
                        )
                    dslc = dst[:, ec, sc * QW : (sc + 1) * QW]
                    if bias is not None:
                        nc.vector.tensor_scalar_add(dslc, ps, bias[:, ec : ec + 1])
                    else:
                        nc.scalar.copy(dslc, ps)

        # ---- V projection: out[s, e] = sum_d v[s, d] W[e, d] + bv ----
        w_sb = wpool.tile([P, DC, D], MDT, tag="w")
        nc.sync.dma_start(out=w_sb, in_=wvT.rearrange("(c p) e -> p c e", p=P))
        vt = instream.tile([P, DC, S], MDT, tag="in")
        nc.sync.dma_start(out=vt, in_=vT.rearrange("(c p) s -> p c s", p=P))
        for sb in range(NQB):
            ps = psmm.tile([P, QW], F32, tag="mm")
            for dc in range(DC):
                nc.tensor.matmul(
                    ps,
                    vt[:, dc, sb * P : (sb + 1) * P],
                    w_sb[:, dc, :],
                    start=(dc == 0),
                    stop=(dc == DC - 1),
                )
            nc.vector.tensor_add(v_sb[:, sb, :], ps, bias_vb)

        # ---- attention, per 512-wide q chunk ----
        inv_sqrt_d = float(1.0 / np.sqrt(D))
        for qc in range(NQC):
            nkb = 4 * qc + 4  # causal: k-blocks 0 .. 4qc+3
            pts = []
            for kb in range(nkb):
                t = kb - 4 * qc  # >=0 only for the diagonal group
                off = max(0, t) * P  # columns below the diagonal are never read
                ps = psmm.tile([P, QW], F32, tag="mm")
                for ec in range(EB):
                    nc.tensor.matmul(
                        ps[:, off:],
                        kt_sb[:, ec, kb * P : (kb + 1) * P],
                        qt_sb[:, ec, qc * QW + off : (qc + 1) * QW],
                        start=(ec == 0),
                        stop=(ec == EB - 1),
                    )
                pt = ptpool.tile([P, QW], MDT, tag="pt")
                nc.scalar.activation(pt[:, off:], ps[:, off:], AF.Exp, scale=inv_sqrt_d)
                if t >= 0:  # diagonal block: mask its triangular 128x128 sub-tile
                    nc.vector.tensor_mul(
                        pt[:, off : off + P], pt[:, off : off + P], cmask
                    )
                pts.append(pt)
            og = opool.tile([P, 4, D], F32, tag="ot")
            for j in range(4):
                qb = 4 * qc + j
                po = psout.tile([P, D], F32, tag="po")
                pr = psrow.tile([P, 2], F32, tag="pr")
                for kb in range(qb + 1):
                    lhsT = pts[kb][:, j * P : (j + 1) * P]
                    nc.tensor.matmul(
                        po, lhsT, v_sb[:, kb, :],
                        start=(kb == 0), stop=(kb == qb),
                    )
                    nc.tensor.matmul(
                        pr, lhsT, ones,
                        start=(kb == 0), stop=(kb == qb),
                    )
                rec = small.tile([P, 1], F32, tag="rec")
                nc.vector.reciprocal(rec, pr[:, 0:1])
                nc.vector.tensor_scalar_mul(og[:, j, :], po, rec)
            nc.sync.dma_start(
                out=out_d[qc * QW : (qc + 1) * QW, :].rearrange(
                    "(c p) e -> p c e", p=P
                ),
                in_=og,
            )

    nc.compile()
    return nc


def _get_nc():
    if "nc" not in _CACHE:
        _CACHE["nc"] = _build()
    return _CACHE["nc"]


def _make_in_maps(q, k, v, Wq, bq, Wk, Wv, bv):
    import ml_dtypes

    mdt = ml_dtypes.bfloat16 if MM_DTYPE == "bf16" else np.float32
    q = np.asarray(q, dtype=np.float32)
    k = np.asarray(k, dtype=np.float32)
    v = np.asarray(v, dtype=np.float32)
    wq_t = np.ascontiguousarray(np.asarray(Wq, dtype=np.float32).T).astype(mdt)
    wk_t = np.ascontiguousarray(np.asarray(Wk, dtype=np.float32).T).astype(mdt)
    wv_t = np.ascontiguousarray(np.asarray(Wv, dtype=np.float32).T).astype(mdt)
    bq4 = np.ascontiguousarray(np.asarray(bq, dtype=np.float32).reshape(EB, P).T)
    bvb = np.ascontiguousarray(
        np.tile(np.asarray(bv, dtype=np.float32)[None, :], (P, 1))
    )
    cm = np.triu(np.ones((P, P), dtype=np.float32)).astype(mdt)  # cm[kk,qq]=qq>=kk
    in_maps = []
    for c in range(N_CORES):
        in_maps.append(
            {
                "qT": np.ascontiguousarray(q[c].T).astype(mdt),
                "kT": np.ascontiguousarray(k[c].T).astype(mdt),
                "vT": np.ascontiguousarray(v[c].T).astype(mdt),
                "wqT": wq_t,
                "wkT": wk_t,
                "wvT": wv_t,
                "bq4": bq4,
                "bvb": bvb,
                "cm": cm,
                "ones_in": np.ones((P, 2), dtype=mdt),
            }
        )
    return in_maps


def _run(in_maps, trace=False):
    from concourse.bass_utils import run_bass_kernel_spmd

    nc = _get_nc()
    res = run_bass_kernel_spmd(
        nc, in_maps, core_ids=list(range(N_CORES)), trace=trace
    )
    out = np.stack([res.results[c]["out"] for c in range(N_CORES)], axis=0)
    return out, res


def kernel(q, k, v, mask, Wq, bq, Wk, bk, Wv, bv):
    q = np.asarray(q, dtype=np.float32)
    assert q.shape == (B, S, D), f"unexpected q shape {q.shape}"
    in_maps = _make_in_maps(q, k, v, Wq, bq, Wk, Wv, bv)
    out, _ = _run(in_maps, trace=False)
    return out


# revision 12
# speedup vs baseline: 1.5049x; 1.0352x over previous
"""Causal single-head attention (B=8, S=2048, D=512) on 8 TRN2 NeuronCores.

Strategy: data-parallel over the batch dim — one batch element per core.
Each core computes, for its batch element:
    Q = q @ Wq.T + bq ; K_core = k @ Wk.T ; V = v @ Wv.T + bv
    scores = Q @ K_core.T / sqrt(D)   (causal)
    out = softmax(scores) @ V
Notes on the math vs the reference:
  - bk drops out exactly: adding bk to K shifts every score row by a
    per-row constant, and softmax is invariant to per-row shifts.
  - softmax is computed without max-subtraction: scores are O(+-6) here
    so exp() cannot overflow/underflow in fp32.
  - bv is folded into the V projection; with late normalization
    out = (P_unnorm @ V) * (1/rowsum), the bias passes through exactly
    because rowsum is computed from the same unnormalized P.
On-device layout: q/k/v arrive host-pre-transposed as [D, S] so the
contraction dim sits on SBUF partitions; QT/KT live as [e, s], V as
[s, e]; score tiles are computed transposed ([s_k=128, s_q<=512]) so
the P tiles feed the PV matmul as stationary operands directly. Row
sums come from an extra N=2 matmul against a ones vector. Only the
lower-triangular 128-column blocks are computed; the 16 diagonal
128x128 sub-tiles are masked with a precomputed triangular 0/1 mask.
Matmul operands are bf16 (PSUM accumulation and the softmax
normalization stay fp32).
"""

import numpy as np

B, S, D, P = 8, 2048, 512, 128
EB = D // P  # e-blocks (4)
DC = D // P  # d-chunks (4)
NQB = S // P  # 128-row q-blocks (16)
QW = 512  # q window (score-tile free dim)
NQC = S // QW  # q-chunks (4)
N_CORES = 8
MM_DTYPE = "bf16"  # "bf16" | "f32r" — dtype of all matmul operands

_CACHE = {}


def _build(causal=True):
    import concourse.tile as tile
    from concourse import bacc, mybir
    from contextlib import ExitStack

    F32 = mybir.dt.float32
    MDT = mybir.dt.bfloat16 if MM_DTYPE == "bf16" else mybir.dt.float32r
    AF = mybir.ActivationFunctionType

    nc = bacc.Bacc("TRN2", target_bir_lowering=False, debug=False)

    qT = nc.dram_tensor("qT", [D, S], MDT, kind="ExternalInput").ap()
    kT = nc.dram_tensor("kT", [D, S], MDT, kind="ExternalInput").ap()
    vT = nc.dram_tensor("vT", [D, S], MDT, kind="ExternalInput").ap()
    wqT = nc.dram_tensor("wqT", [D, D], MDT, kind="ExternalInput").ap()
    wkT = nc.dram_tensor("wkT", [D, D], MDT, kind="ExternalInput").ap()
    wvT = nc.dram_tensor("wvT", [D, D], MDT, kind="ExternalInput").ap()
    bq4 = nc.dram_tensor("bq4", [P, EB], F32, kind="ExternalInput").ap()
    bvb = nc.dram_tensor("bvb", [P, D], F32, kind="ExternalInput").ap()
    cm = nc.dram_tensor("cm", [P, P], MDT, kind="ExternalInput").ap()
    ones_d = nc.dram_tensor("ones_in", [P, 2], MDT, kind="ExternalInput").ap()
    out_d = nc.dram_tensor("out", [S, D], F32, kind="ExternalOutput").ap()

    with tile.TileContext(nc) as tc, ExitStack() as ctx:
        consts = ctx.enter_context(tc.tile_pool(name="consts", bufs=1))
        wpool = ctx.enter_context(tc.tile_pool(name="wpool", bufs=2))
        instream = ctx.enter_context(tc.tile_pool(name="instream", bufs=3))
        acts = ctx.enter_context(tc.tile_pool(name="acts", bufs=1))
        ptpool = ctx.enter_context(tc.tile_pool(name="ptpool", bufs=18))
        opool = ctx.enter_context(tc.tile_pool(name="opool", bufs=2))
        small = ctx.enter_context(tc.tile_pool(name="small", bufs=4))
        psmm = ctx.enter_context(tc.tile_pool(name="psmm", bufs=4, space="PSUM"))
        psout = ctx.enter_context(tc.tile_pool(name="psout", bufs=2, space="PSUM"))
        psrow = ctx.enter_context(tc.tile_pool(name="psrow", bufs=2, space="PSUM"))

        cmask = consts.tile([P, P], MDT)
        bias_q = consts.tile([P, EB], F32)
        bias_vb = consts.tile([P, D], F32)
        ones = consts.tile([P, 2], MDT)

        # persistent per-core activations
        qt_sb = acts.tile([P, EB, S], MDT, tag="qt")  # QT[e, s]
        kt_sb = acts.tile([P, EB, S], MDT, tag="kt")  # KT[e, s] (no bias)
        v_sb = acts.tile([P, NQB, D], MDT, tag="v")  # V[s, e] (+bv)

        # ---- Q/K projections: out[e, s] = sum_d W[e, d] x[s, d] (+ bias) ----
        for w_dram, x_dram, dst, bias in (
            (wqT, qT, qt_sb, bias_q),
            (wkT, kT, kt_sb, None),
        ):
            w_sb = wpool.tile([P, DC, D], MDT, tag="w")
            w_r = w_dram.rearrange("(c p) e -> p c e", p=P)
            x_r = x_dram.rearrange("(c p) s -> p c s", p=P)
            xt = instream.tile([P, DC, S], MDT, tag="in")
            nc.scalar.dma_start(out=w_sb, in_=w_r)
            for sc in range(NQC):  # per-window pieces arrive as PE consumes them
                if bias is not None and sc == 0:
                    # split the first piece across both HWDGE queues
                    nc.sync.dma_start(out=xt[:, :2, :QW], in_=x_r[:, :2, :QW])
                    nc.scalar.dma_start(out=xt[:, 2:, :QW], in_=x_r[:, 2:, :QW])
                    continue
                nc.sync.dma_start(
                    out=xt[:, :, sc * QW : (sc + 1) * QW],
                    in_=x_r[:, :, sc * QW : (sc + 1) * QW],
                )
            if bias is not None:
                # consts queued after the critical Q pieces
                nc.scalar.dma_start(out=cmask, in_=cm)
                nc.scalar.dma_start(out=bias_q, in_=bq4)
                nc.scalar.dma_start(out=bias_vb, in_=bvb)
                nc.scalar.dma_start(out=ones, in_=ones_d)
            for sc in range(NQC):
                for ec in range(EB):
                    ps = psmm.tile([P, QW], F32, tag="mm")
                    for dc in range(DC):
                        nc.tensor.matmul(
                            ps,
                            w_sb[:, dc, ec * P : (ec + 1) * P],
                            xt[:, dc, sc * QW : (sc + 1) * QW],
                            start=(dc == 0),
                            stop=(dc == DC - 1),
                        )
                    dslc = dst[:, ec, sc * QW : (sc + 1) * QW]
                    if bias is not None:
                        nc.vector.tensor_scalar_add(dslc, ps, bias[:, ec : ec + 1])
                    else:
                        nc.scalar.copy(dslc, ps)

        # ---- V projection: out[s, e] = sum_d v[s, d] W[e, d] + bv ----
        w_sb = wpool.tile([P, DC, D], MDT, tag="w")
        nc.scalar.dma_start(out=w_sb, in_=wvT.rearrange("(c p) e -> p c e", p=P))
        vt = instream.tile([P, DC, S], MDT, tag="in")
        nc.sync.dma_start(out=vt, in_=vT.rearrange("(c p) s -> p c s", p=P))
        for sb in range(NQB):
            ps = psmm.tile([P, QW], F32, tag="mm")
            for dc in range(DC):
                nc.tensor.matmul(
                    ps,
                    vt[:, dc, sb * P : (sb + 1) * P],
                    w_sb[:, dc, :],
                    start=(dc == 0),
                    stop=(dc == DC - 1),
                )
            nc.vector.tensor_add(v_sb[:, sb, :], ps, bias_vb)

        # ---- attention, per 512-wide q chunk ----
        inv_sqrt_d = float(1.0 / np.sqrt(D))
        for qc in range(NQC):
            nkb = 4 * qc + 4 if causal else NQB  # causal: k-blocks 0..4qc+3
            pts = []
            for kb in range(nkb):
                t = kb - 4 * qc if causal else -1  # >=0: diagonal group
                off = max(0, t) * P  # columns below the diagonal are never read
                ps = psmm.tile([P, QW], F32, tag="mm")
                for ec in range(EB):
                    nc.tensor.matmul(
                        ps[:, off:],
                        kt_sb[:, ec, kb * P : (kb + 1) * P],
                        qt_sb[:, ec, qc * QW + off : (qc + 1) * QW],
                        start=(ec == 0),
                        stop=(ec == EB - 1),
                    )
                pt = ptpool.tile([P, QW], MDT, tag="pt")
                nc.scalar.activation(pt[:, off:], ps[:, off:], AF.Exp, scale=inv_sqrt_d)
                if t >= 0:  # diagonal block: mask its triangular 128x128 sub-tile
                    nc.vector.tensor_mul(
                        pt[:, off : off + P], pt[:, off : off + P], cmask
                    )
                pts.append(pt)
            og = opool.tile([P, 4, D], F32, tag="ot")
            for j in range(4):
                qb = 4 * qc + j
                po = psout.tile([P, D], F32, tag="po")
                pr = psrow.tile([P, 2], F32, tag="pr")
                kb_hi = qb if causal else NQB - 1
                for kb in range(kb_hi + 1):
                    lhsT = pts[kb][:, j * P : (j + 1) * P]
                    nc.tensor.matmul(
                        po, lhsT, v_sb[:, kb, :],
                        start=(kb == 0), stop=(kb == kb_hi),
                    )
                    nc.tensor.matmul(
                        pr, lhsT, ones,
                        start=(kb == 0), stop=(kb == kb_hi),
                    )
                rec = small.tile([P, 1], F32, tag="rec")
                nc.vector.reciprocal(rec, pr[:, 0:1])
                nc.vector.tensor_scalar_mul(og[:, j, :], po, rec)
                nc.sync.dma_start(
                    out=out_d[qb * P : (qb + 1) * P, :], in_=og[:, j, :]
                )

    nc.compile()
    return nc


def _get_nc(causal=True):
    key = ("nc", causal)
    if key not in _CACHE:
        _CACHE[key] = _build(causal)
    return _CACHE[key]


def _make_in_maps(q, k, v, Wq, bq, Wk, Wv, bv):
    import ml_dtypes

    mdt = ml_dtypes.bfloat16 if MM_DTYPE == "bf16" else np.float32
    q = np.asarray(q, dtype=np.float32)
    k = np.asarray(k, dtype=np.float32)
    v = np.asarray(v, dtype=np.float32)
    wq_t = np.ascontiguousarray(np.asarray(Wq, dtype=np.float32).T).astype(mdt)
    wk_t = np.ascontiguousarray(np.asarray(Wk, dtype=np.float32).T).astype(mdt)
    wv_t = np.ascontiguousarray(np.asarray(Wv, dtype=np.float32).T).astype(mdt)
    bq4 = np.ascontiguousarray(np.asarray(bq, dtype=np.float32).reshape(EB, P).T)
    bvb = np.ascontiguousarray(
        np.tile(np.asarray(bv, dtype=np.float32)[None, :], (P, 1))
    )
    cm = np.triu(np.ones((P, P), dtype=np.float32)).astype(mdt)  # cm[kk,qq]=qq>=kk
    in_maps = []
    for c in range(N_CORES):
        in_maps.append(
            {
                "qT": np.ascontiguousarray(q[c].T).astype(mdt),
                "kT": np.ascontiguousarray(k[c].T).astype(mdt),
                "vT": np.ascontiguousarray(v[c].T).astype(mdt),
                "wqT": wq_t,
                "wkT": wk_t,
                "wvT": wv_t,
                "bq4": bq4,
                "bvb": bvb,
                "cm": cm,
                "ones_in": np.ones((P, 2), dtype=mdt),
            }
        )
    return in_maps


def _run(in_maps, trace=False, causal=True):
    from concourse.bass_utils import run_bass_kernel_spmd

    nc = _get_nc(causal)
    res = run_bass_kernel_spmd(
        nc, in_maps, core_ids=list(range(N_CORES)), trace=trace
    )
    out = np.stack([res.results[c]["out"] for c in range(N_CORES)], axis=0)
    return out, res


def _mask_is_causal(mask):
    m = np.asarray(mask).reshape(S, S).astype(bool)
    if m.all():
        return False  # attend-to-everything mask: run the dense variant
    tril = np.tril(np.ones((S, S), dtype=bool))
    if np.array_equal(m, tril):
        return True
    raise ValueError("unsupported mask pattern (expected causal or all-ones)")


def kernel(q, k, v, mask, Wq, bq, Wk, bk, Wv, bv):
    q = np.asarray(q, dtype=np.float32)
    assert q.shape == (B, S, D), f"unexpected q shape {q.shape}"
    causal = _mask_is_causal(mask)
    in_maps = _make_in_maps(q, k, v, Wq, bq, Wk, Wv, bv)
    out, _ = _run(in_maps, trace=False, causal=causal)
    return out


# revision 14
# speedup vs baseline: 1.6039x; 1.0658x over previous
"""Causal single-head attention (B=8, S=2048, D=512) on 8 TRN2 NeuronCores.

Strategy: data-parallel over the batch dim — one batch element per core.
Reference math per batch element:
    Q = q @ Wq.T + bq ; K = k @ Wk.T + bk ; V = v @ Wv.T + bv
    scores = Q @ K.T / sqrt(D)  (causal) ; out = softmax(scores) @ V
Algebra used on device:
  - bk drops out exactly (softmax is invariant to per-row score shifts).
  - The K projection is never materialized: with N^T = Wq^T @ Wk,
        scores^T = k @ (q @ N^T)^T + c 1^T,   c = k @ (Wk^T bq)
    so one big projection H = q @ N^T replaces the Q and K projections,
    and bq enters as the per-key additive constant c, folded into the
    exp() activation's per-partition bias.
  - softmax runs without max-subtraction: scores are O(+-6) here so
    fp32 exp() cannot overflow/underflow.
  - bv is folded into the V projection; with late normalization
    out = (P_unnorm @ V) * (1/rowsum) the bias passes through exactly
    because rowsum comes from the same unnormalized P.
Layout: q/k/v arrive host-pre-arranged as [128, 4, S] (contraction dim
on partitions, contiguous per partition). Score tiles are computed
transposed ([s_k=128, s_q<=512]) so the exp'd P tiles feed the PV
matmul directly as stationary operands. Row sums come from an N=2
matmul against ones. Only lower-triangular 128-col blocks are
computed; the 16 diagonal sub-tiles are masked with a 0/1 triangle.
Matmul operands are bf16; PSUM accumulation / softmax normalization /
output stay fp32. A short dummy-matmul warm-up releases the PE HAM
clock throttle while the first DMAs are in flight.
"""

import numpy as np

B, S, D, P = 8, 2048, 512, 128
EB = D // P  # e-blocks (4)
DC = D // P  # d-chunks (4)
NQB = S // P  # 128-row q-blocks (16)
QW = 512  # q window (score-tile free dim)
NQC = S // QW  # q-chunks (4)
N_CORES = 8
MM_DTYPE = "bf16"  # "bf16" | "f32r" — dtype of all matmul operands

_CACHE = {}


def _build(causal=True):
    import concourse.tile as tile
    from concourse import bacc, mybir
    from contextlib import ExitStack

    F32 = mybir.dt.float32
    MDT = mybir.dt.bfloat16 if MM_DTYPE == "bf16" else mybir.dt.float32r
    AF = mybir.ActivationFunctionType

    nc = bacc.Bacc("TRN2", target_bir_lowering=False, debug=False)

    qT = nc.dram_tensor("qT", [P, DC, S], MDT, kind="ExternalInput").ap()
    kT = nc.dram_tensor("kT", [P, DC, S], MDT, kind="ExternalInput").ap()
    vT = nc.dram_tensor("vT", [P, DC, S], MDT, kind="ExternalInput").ap()
    wqN = nc.dram_tensor("wqN", [P, EB, D], MDT, kind="ExternalInput").ap()
    wkN = nc.dram_tensor("wkN", [P, EB, D], MDT, kind="ExternalInput").ap()
    wvT = nc.dram_tensor("wvT", [P, DC, D], MDT, kind="ExternalInput").ap()
    bq2 = nc.dram_tensor("bq2", [P, EB, 2], MDT, kind="ExternalInput").ap()
    bvb = nc.dram_tensor("bvb", [P, D], F32, kind="ExternalInput").ap()
    cm = nc.dram_tensor("cm", [P, P], MDT, kind="ExternalInput").ap()
    ones_d = nc.dram_tensor("ones_in", [P, 2], MDT, kind="ExternalInput").ap()
    out_d = nc.dram_tensor("out", [S, D], F32, kind="ExternalOutput").ap()

    with tile.TileContext(nc) as tc, ExitStack() as ctx:
        consts = ctx.enter_context(tc.tile_pool(name="consts", bufs=1))
        wpool = ctx.enter_context(tc.tile_pool(name="wpool", bufs=2))
        instream = ctx.enter_context(tc.tile_pool(name="instream", bufs=2))
        acts = ctx.enter_context(tc.tile_pool(name="acts", bufs=1))
        ptpool = ctx.enter_context(tc.tile_pool(name="ptpool", bufs=18))
        opool = ctx.enter_context(tc.tile_pool(name="opool", bufs=2))
        small = ctx.enter_context(tc.tile_pool(name="small", bufs=4))
        psmm = ctx.enter_context(tc.tile_pool(name="psmm", bufs=4, space="PSUM"))
        psout = ctx.enter_context(tc.tile_pool(name="psout", bufs=2, space="PSUM"))
        psrow = ctx.enter_context(tc.tile_pool(name="psrow", bufs=2, space="PSUM"))

        cmask = consts.tile([P, P], MDT)
        bias_vb = consts.tile([P, D], F32)
        ones = consts.tile([P, 2], MDT)
        bqc = consts.tile([P, EB, 2], MDT)

        # PE warm-up: ~3.5us of dummy matmuls releases the HAM clock throttle
        # while the first input DMAs are still in flight.
        warm = consts.tile([P, QW], MDT)
        nc.vector.memset(warm, 0.0)
        wps = psmm.tile([P, QW], F32, tag="mm")
        for _ in range(16):
            nc.tensor.matmul(wps, warm[:, :P], warm, start=True, stop=True)

        # persistent per-core activations
        ht_sb = acts.tile([P, DC, S], MDT, tag="ht")  # H^T[d, s] = N^T q^T
        kin = acts.tile([P, DC, S], MDT, tag="kin")  # k^T input (resident)
        v_sb = acts.tile([P, NQB, D], MDT, tag="v")  # V[s, e] (+bv)
        nt_sb = acts.tile([P, DC, D], MDT, tag="nt")  # N^T[d2, d1] = Wq^T Wk
        u_sb = acts.tile([P, DC, 2], MDT, tag="u")  # u[d] = Wk^T bq
        c_sb = consts.tile([P, NQB], F32)  # c/sqrt(D) per key block

        # ---- DMAs: weights on the scalar HWDGE queue, inputs on sync ----
        wq_sb = wpool.tile([P, EB, D], MDT, tag="w")
        wk_sb = wpool.tile([P, EB, D], MDT, tag="w")
        nc.scalar.dma_start(out=wq_sb, in_=wqN)
        nc.scalar.dma_start(out=wk_sb, in_=wkN)
        qt_in = instream.tile([P, DC, S], MDT, tag="in")
        half = S // 2
        nc.sync.dma_start(out=qt_in[:, :, :half], in_=qT[:, :, :half])
        nc.sync.dma_start(out=qt_in[:, :, half:], in_=qT[:, :, half:])
        nc.scalar.dma_start(out=bqc, in_=bq2)
        nc.scalar.dma_start(out=cmask, in_=cm)
        nc.scalar.dma_start(out=bias_vb, in_=bvb)
        nc.scalar.dma_start(out=ones, in_=ones_d)
        nc.sync.dma_start(out=kin, in_=kT)

        # ---- N^T = Wq^T Wk  and  u = Wk^T bq ----
        for d2c in range(DC):
            ps = psmm.tile([P, QW], F32, tag="mm")
            for ec in range(EB):
                nc.tensor.matmul(
                    ps,
                    wq_sb[:, ec, d2c * P : (d2c + 1) * P],
                    wk_sb[:, ec, :],
                    start=(ec == 0),
                    stop=(ec == EB - 1),
                )
            nc.vector.tensor_copy(nt_sb[:, d2c, :], ps)
        for dc in range(DC):
            pu = psrow.tile([P, 2], F32, tag="pr")
            for ec in range(EB):
                nc.tensor.matmul(
                    pu,
                    wk_sb[:, ec, dc * P : (dc + 1) * P],
                    bqc[:, ec, :],
                    start=(ec == 0),
                    stop=(ec == EB - 1),
                )
            nc.vector.tensor_copy(u_sb[:, dc, :], pu)

        # ---- H^T = N^T q^T  (the single big projection) ----
        for sc in range(NQC):
            for dcm in range(DC):
                ps = psmm.tile([P, QW], F32, tag="mm")
                for dpc in range(DC):
                    nc.tensor.matmul(
                        ps,
                        nt_sb[:, dpc, dcm * P : (dcm + 1) * P],
                        qt_in[:, dpc, sc * QW : (sc + 1) * QW],
                        start=(dpc == 0),
                        stop=(dpc == DC - 1),
                    )
                nc.scalar.copy(ht_sb[:, dcm, sc * QW : (sc + 1) * QW], ps)

        # ---- c = k u  (per-key score constant from bq), pre-scaled ----
        inv_sqrt_d = float(1.0 / np.sqrt(D))
        for kb in range(NQB):
            pc = psrow.tile([P, 2], F32, tag="pr")
            for dc in range(DC):
                nc.tensor.matmul(
                    pc,
                    kin[:, dc, kb * P : (kb + 1) * P],
                    u_sb[:, dc, :],
                    start=(dc == 0),
                    stop=(dc == DC - 1),
                )
            nc.vector.tensor_scalar_mul(c_sb[:, kb : kb + 1], pc[:, 0:1], inv_sqrt_d)

        # ---- V projection: out[s, e] = sum_d v[s, d] W[e, d] + bv ----
        wv_sb = wpool.tile([P, DC, D], MDT, tag="w")
        nc.scalar.dma_start(out=wv_sb, in_=wvT)
        vt = instream.tile([P, DC, S], MDT, tag="in")
        nc.sync.dma_start(out=vt, in_=vT)
        for sb in range(NQB):
            ps = psmm.tile([P, QW], F32, tag="mm")
            for dc in range(DC):
                nc.tensor.matmul(
                    ps,
                    vt[:, dc, sb * P : (sb + 1) * P],
                    wv_sb[:, dc, :],
                    start=(dc == 0),
                    stop=(dc == DC - 1),
                )
            nc.vector.tensor_add(v_sb[:, sb, :], ps, bias_vb)

        # ---- attention, per 512-wide q chunk ----
        for qc in range(NQC):
            nkb = 4 * qc + 4 if causal else NQB  # causal: k-blocks 0..4qc+3
            pts = []
            for kb in range(nkb):
                t = kb - 4 * qc if causal else -1  # >=0: diagonal group
                off = max(0, t) * P  # columns below the diagonal are never read
                ps = psmm.tile([P, QW], F32, tag="mm")
                for dc in range(DC):
                    nc.tensor.matmul(
                        ps[:, off:],
                        kin[:, dc, kb * P : (kb + 1) * P],
                        ht_sb[:, dc, qc * QW + off : (qc + 1) * QW],
                        start=(dc == 0),
                        stop=(dc == DC - 1),
                    )
                pt = ptpool.tile([P, QW], MDT, tag="pt")
                nc.scalar.activation(
                    pt[:, off:], ps[:, off:], AF.Exp,
                    bias=c_sb[:, kb : kb + 1], scale=inv_sqrt_d,
                )
                if t >= 0:  # diagonal block: mask its triangular 128x128 sub-tile
                    nc.vector.tensor_mul(
                        pt[:, off : off + P], pt[:, off : off + P], cmask
                    )
                pts.append(pt)
            og = opool.tile([P, 4, D], F32, tag="ot")
            for j in range(4):
                qb = 4 * qc + j
                po = psout.tile([P, D], F32, tag="po")
                pr = psrow.tile([P, 2], F32, tag="pr")
                kb_hi = qb if causal else NQB - 1
                for kb in range(kb_hi + 1):
                    lhsT = pts[kb][:, j * P : (j + 1) * P]
                    nc.tensor.matmul(
                        po, lhsT, v_sb[:, kb, :],
                        start=(kb == 0), stop=(kb == kb_hi),
                    )
                    nc.tensor.matmul(
                        pr, lhsT, ones,
                        start=(kb == 0), stop=(kb == kb_hi),
                    )
                rec = small.tile([P, 1], F32, tag="rec")
                nc.vector.reciprocal(rec, pr[:, 0:1])
                nc.vector.tensor_scalar_mul(og[:, j, :], po, rec)
                nc.sync.dma_start(
                    out=out_d[qb * P : (qb + 1) * P, :], in_=og[:, j, :]
                )

    nc.compile()
    return nc


def _get_nc(causal=True):
    key = ("nc", causal)
    if key not in _CACHE:
        _CACHE[key] = _build(causal)
    return _CACHE[key]


def _make_in_maps(q, k, v, Wq, bq, Wk, Wv, bv):
    import ml_dtypes

    mdt = ml_dtypes.bfloat16 if MM_DTYPE == "bf16" else np.float32
    q = np.asarray(q, dtype=np.float32)
    k = np.asarray(k, dtype=np.float32)
    v = np.asarray(v, dtype=np.float32)

    def wnat(w):  # [e, d] -> [p, ec, d] with e = ec*P + p
        wn = np.asarray(w, dtype=np.float32).reshape(EB, P, D)
        return np.ascontiguousarray(wn.transpose(1, 0, 2)).astype(mdt)

    def warr(w):  # [e, d] -> [p, dc, e] with d = dc*P + p
        wt = np.asarray(w, dtype=np.float32).T.reshape(DC, P, D)
        return np.ascontiguousarray(wt.transpose(1, 0, 2)).astype(mdt)

    def xarr(x):  # [s, d] -> [p, dc, s] with d = dc*P + p
        xt = np.ascontiguousarray(x.T).reshape(DC, P, S)
        return np.ascontiguousarray(xt.transpose(1, 0, 2)).astype(mdt)

    wq_n = wnat(Wq)
    wk_n = wnat(Wk)
    wv_t = warr(Wv)
    bq_f = np.asarray(bq, dtype=np.float32).reshape(EB, P).T  # [P, EB]
    bq2 = np.ascontiguousarray(
        np.repeat(bq_f[:, :, None], 2, axis=2)
    ).astype(mdt)  # [P, EB, 2]
    bvb = np.ascontiguousarray(
        np.tile(np.asarray(bv, dtype=np.float32)[None, :], (P, 1))
    )
    cm = np.triu(np.ones((P, P), dtype=np.float32)).astype(mdt)  # cm[kk,qq]=qq>=kk
    in_maps = []
    for c in range(N_CORES):
        in_maps.append(
            {
                "qT": xarr(q[c]),
                "kT": xarr(k[c]),
                "vT": xarr(v[c]),
                "wqN": wq_n,
                "wkN": wk_n,
                "wvT": wv_t,
                "bq2": bq2,
                "bvb": bvb,
                "cm": cm,
                "ones_in": np.ones((P, 2), dtype=mdt),
            }
        )
    return in_maps


def _run(in_maps, trace=False, causal=True):
    from concourse.bass_utils import run_bass_kernel_spmd

    nc = _get_nc(causal)
    res = run_bass_kernel_spmd(
        nc, in_maps, core_ids=list(range(N_CORES)), trace=trace
    )
    out = np.stack([res.results[c]["out"] for c in range(N_CORES)], axis=0)
    return out, res


def _mask_is_causal(mask):
    m = np.asarray(mask).reshape(S, S).astype(bool)
    if m.all():
        return False  # attend-to-everything mask: run the dense variant
    tril = np.tril(np.ones((S, S), dtype=bool))
    if np.array_equal(m, tril):
        return True
    raise ValueError("unsupported mask pattern (expected causal or all-ones)")


def kernel(q, k, v, mask, Wq, bq, Wk, bk, Wv, bv):
    q = np.asarray(q, dtype=np.float32)
    assert q.shape == (B, S, D), f"unexpected q shape {q.shape}"
    causal = _mask_is_causal(mask)
    in_maps = _make_in_maps(q, k, v, Wq, bq, Wk, Wv, bv)
    out, _ = _run(in_maps, trace=False, causal=causal)
    return out
